# revision 1
# baseline (speedup 1.0000x reference)
"""Trainium2 Bass kernel for nn_BlipAttention_75007308857568.

Single-head BLIP attention: B=32, N=1024, C=768, fp32.
  qkv = x @ qkv_w + qkv_b ; q,k,v split
  scores = q @ k.T / sqrt(C) ; attn = softmax(scores)
  out = attn @ v
  y = (out.swapaxes(1,2).reshape(B,N,C)) @ proj_w + proj_b

Sharding: data-parallel over batch B across 8 NeuronCores (4 batches/core).

Per-core dataflow (transposed domain keeps contraction dims on SBUF
partitions):
  XT  = x[b].T                       (PE transposes, f32r identity ->
                                      1.5 cyc/row instead of fp32's 2)
  QT/KT = (Wq|Wk).T @ XT             (PE fp32r; per-partition qkv bias added
                                      on the ACT engine, output written
                                      directly as fp8e4)
  V   = x[b] @ Wv + v_bias           (PE fp32r; bias add + fp8e4 convert on
                                      the Pool engine)
  scoresT[m,n] = KT.T@QT             (PE fp8 DoubleRow: 256-deep contraction
                                      per instr at 0.5 cyc/row)
  expT = exp(scoresT/sqrt(C) - 2)    (ACT, PSUM->SBUF fp8e4; the -2 shift
                                      keeps exp <= e^4.8 < 240 = trn2 e4m3
                                      max; softmax is shift-invariant)
  denom = ones.T @ expT              (PE fp8 DoubleRow)
  OT[c,n] = (V.T @ expT) * recip     (PE fp8 DoubleRow + DVE normalize)
  scratch flat = OT (c-major)        -> flat viewed as [N,C] IS the
                                       swapaxes+reshape permutation for free
  PT = transpose(P rows)             (PE, bf16 identity)
  y = P @ proj_w + proj_b            (PE fp32r; bias add on Pool engine)

Engine balance: PE is the bottleneck; ACT takes the qk bias adds and exp
(per-partition bias), DVE takes everything else PSUM-sourced (GPSIMD/Pool
cannot access PSUM). Engine queues are in-order, so next-batch QKV matmuls
are interleaved into the attention emission stream to cover the PE bubbles
that the scores->exp->denom/AV dependency chain would otherwise create.
"""

import math
import os

import numpy as np

import concourse.bacc as bacc
import concourse.bass as bass
import concourse.mybir as mybir
import concourse.tile as tile

from concourse.bass_utils import run_bass_kernel_spmd
from concourse.masks import make_identity

B, N, C = 32, 1024, 768
NCORES = 8
BPC = B // NCORES  # batches per core
CB = C // 128      # 6 channel blocks
NB = N // 128      # 8 sequence blocks
NH = 512           # n-half width (PSUM bank / fp32 moving-operand limit)
SCALE = 1.0 / math.sqrt(C)
EXPB = -2.0        # exp shift (max logit ~6.73 -> exp(4.73)=113 < 240)

_CACHE = {}


def _build(mm_r: bool, fp8: bool):
    dt = mybir.dt
    MM = dt.float32r if mm_r else dt.float32
    AT = dt.float8e4 if fp8 else dt.bfloat16  # attention-core operand dtype
    f32 = dt.float32
    DR = mybir.MatmulPerfMode.DoubleRow if fp8 else None

    nc = bacc.Bacc("TRN2", target_bir_lowering=False, debug=False)

    # xs/scr are declared in the matmul dtype (f32r == f32 bits) so the PE
    # transposes can pair with the bf16 identity (1 cyc/row, not fp32's 2)
    xs = nc.dram_tensor("xs", [BPC, N, C], MM, kind="ExternalInput")
    qkv_w = nc.dram_tensor("qkv_w", [C, 3 * C], MM, kind="ExternalInput")
    qkv_b = nc.dram_tensor("qkv_b", [3 * C], f32, kind="ExternalInput")
    proj_w = nc.dram_tensor("proj_w", [C, C], MM, kind="ExternalInput")
    proj_b = nc.dram_tensor("proj_b", [C], f32, kind="ExternalInput")
    y = nc.dram_tensor("y", [BPC, N, C], f32, kind="ExternalOutput")

    with tile.TileContext(nc) as tc:
        with (
            tc.tile_pool(name="consts", bufs=1) as consts,
            tc.tile_pool(name="xt", bufs=1) as pool_xt,
            tc.tile_pool(name="qt", bufs=2) as pool_qt,
            tc.tile_pool(name="kt", bufs=2) as pool_kt,
            tc.tile_pool(name="v", bufs=1) as pool_v,
            tc.tile_pool(name="expt", bufs=1) as pool_expt,
            tc.tile_pool(name="row", bufs=4) as pool_row,
            tc.tile_pool(name="pt", bufs=2) as pool_pt,
            tc.tile_pool(name="rb", bufs=2) as pool_rb,
            tc.tile_pool(name="scr", bufs=2, space="DRAM") as pool_scr,
            tc.tile_pool(name="psmm", bufs=6, space="PSUM") as psmm,
            tc.tile_pool(name="pst", bufs=2, space="PSUM") as pst,
        ):
            # ---- constants / weights (loaded once) ----
            # f32r identity: transpose cost keys on the moving operand (the
            # identity) — 1.5 cyc/row vs fp32's 2.0. bf16 would be 1.0 but
            # walrus forbids mixing 32-bit data with non-32-bit identity.
            # Built as f32 (memset/affine_select can't emit f32r), then
            # copied through DVE so the result is "rounded to f32r" as the
            # BIR verifier requires of f32r matmult inputs.
            ident_f = consts.tile([128, 128], f32, tag="ident_f")
            make_identity(nc, ident_f)
            ident = consts.tile([128, 128], MM, tag="ident")
            nc.vector.tensor_copy(ident, ident_f)

            W = consts.tile([128, CB, 3 * C], MM, tag="W")
            PW = consts.tile([128, CB, C], MM, tag="PW")

            # q/k bias as per-partition scalars: qkb[p, ob] = qkv_b[ob*128+p]
            qkb = consts.tile([128, 2 * CB], f32, tag="qkb")
            nc.sync.dma_start(
                qkb, qkv_b.ap()[0 : 2 * C].rearrange("(ob p) -> p ob", p=128)
            )
            # v bias / proj bias replicated across partitions
            vb = consts.tile([128, C], f32, tag="vb")
            nc.sync.dma_start(vb, qkv_b.ap()[None, 2 * C : 3 * C].to_broadcast([128, C]))
            pb = consts.tile([128, C], f32, tag="pb")
            nc.sync.dma_start(pb, proj_b.ap()[None, :].to_broadcast([128, C]))

            expb = consts.tile([128, 1], f32, tag="expb")
            nc.gpsimd.memset(expb, EXPB)

            # all-ones stationary operand for the softmax denominator matmul
            # (replicates the column sums of expT onto all 128 partitions)
            ones_f = consts.tile([128, 256 if fp8 else 128], f32, tag="ones_f")
            nc.gpsimd.memset(ones_f, 1.0)
            ones = consts.tile([128, 256 if fp8 else 128], AT, tag="ones")
            nc.gpsimd.tensor_copy(ones, ones_f)

            def transpose_block(src_row, dst_slices):
                """PE-transpose six 128x128 chunks of src_row, batched 4+2
                per PSUM bank, with one grouped DVE copy per bank."""
                psA = pst.tile([128, NH], MM, tag="tp4")
                for k in range(4):
                    nc.tensor.transpose(
                        psA[:, k * 128 : (k + 1) * 128],
                        src_row[:, k * 128 : (k + 1) * 128],
                        ident,
                    )
                nc.vector.tensor_copy(
                    dst_slices[0], psA.rearrange("p (c k) -> p c k", k=128)
                )
                psB = pst.tile([128, NH], MM, tag="tp4")
                for k in range(2):
                    nc.tensor.transpose(
                        psB[:, k * 128 : (k + 1) * 128],
                        src_row[:, (4 + k) * 128 : (5 + k) * 128],
                        ident,
                    )
                nc.vector.tensor_copy(
                    dst_slices[1],
                    psB[:, 0:256].rearrange("p (c k) -> p c k", k=128),
                )

            def emit_a_row_dma(b, nb):
                xrow = pool_row.tile([128, C], MM, tag="row")
                nc.sync.dma_start(xrow, xs.ap()[b, nb * 128 : (nb + 1) * 128, :])
                return xrow

            def emit_a_row_transpose(XT, xrow, nb):
                nsl = slice(nb * 128, (nb + 1) * 128)
                transpose_block(xrow, [XT[:, 0:4, nsl], XT[:, 4:6, nsl]])

            def stage_a(b):
                """XT = x[b].T"""
                XT = pool_xt.tile([128, CB, N], MM, tag="XT")
                for nb in range(NB):
                    xrow = emit_a_row_dma(b, nb)
                    emit_a_row_transpose(XT, xrow, nb)
                return XT

            def emit_qk_tile(XT, QT, KT, ob, nh):
                """One q/k output tile: 6 fp32r matmuls + ACT bias-add that
                writes the fp8 (or f32r) QT/KT slice."""
                dest = QT if ob < CB else KT
                dcb = ob % CB
                ps = psmm.tile([128, NH], f32, tag="mm")
                for cb in range(CB):
                    nc.tensor.matmul(
                        ps,
                        W[:, cb, ob * 128 : (ob + 1) * 128],
                        XT[:, cb, nh * NH : (nh + 1) * NH],
                        start=(cb == 0),
                        stop=(cb == CB - 1),
                    )
                nc.scalar.add(
                    dest[:, dcb, nh * NH : (nh + 1) * NH], ps, qkb[:, ob : ob + 1]
                )

            def emit_v_chunk(XT, V, mb):
                """V[mb] = x[mb-block] @ Wv + v_bias (DVE: only DVE/ACT can
                read PSUM, and the bias varies along the free dim)."""
                for c0, cw in ((0, NH), (NH, C - NH)):
                    ps = psmm.tile([128, NH], f32, tag="mm")
                    for cb in range(CB):
                        nc.tensor.matmul(
                            ps[:, :cw],
                            XT[:, cb, mb * 128 : (mb + 1) * 128],
                            W[:, cb, 2 * C + c0 : 2 * C + c0 + cw],
                            start=(cb == 0),
                            stop=(cb == CB - 1),
                        )
                    nc.vector.tensor_tensor(
                        V[:, mb, c0 : c0 + cw], ps[:, :cw],
                        vb[:, c0 : c0 + cw], op=mybir.AluOpType.add,
                    )

            def emit_scores_tile(QT, KT, expT, nh, mb):
                """scoresT tile [m-block, n-half] + exp on ACT -> fp8 expT."""
                nsl = slice(nh * NH, (nh + 1) * NH)
                msl = slice(mb * 128, (mb + 1) * 128)
                ps = psmm.tile([128, NH], f32, tag="mm")
                if fp8:
                    for p in range(CB // 2):
                        nc.tensor.matmul(
                            ps,
                            KT[:, 2 * p : 2 * p + 2, msl],
                            QT[:, 2 * p : 2 * p + 2, nsl],
                            start=(p == 0),
                            stop=(p == CB // 2 - 1),
                            perf_mode=DR,
                        )
                else:
                    for cb in range(CB):
                        nc.tensor.matmul(
                            ps, KT[:, cb, msl], QT[:, cb, nsl],
                            start=(cb == 0), stop=(cb == CB - 1),
                        )
                nc.scalar.activation(
                    expT[:, mb, nsl], ps, mybir.ActivationFunctionType.Exp,
                    scale=SCALE, bias=expb[:, 0:1],
                )

            def emit_denom(expT, nh):
                """denominator (replicated on all partitions) for one n-half"""
                nsl = slice(nh * NH, (nh + 1) * NH)
                dps = psmm.tile([128, NH], f32, tag="mm")
                if fp8:
                    ones_v = ones.rearrange("p (k f) -> p k f", k=2)
                    for p in range(NB // 2):
                        nc.tensor.matmul(
                            dps, ones_v, expT[:, 2 * p : 2 * p + 2, nsl],
                            start=(p == 0), stop=(p == NB // 2 - 1),
                            perf_mode=DR,
                        )
                else:
                    for mb in range(NB):
                        nc.tensor.matmul(
                            dps, ones, expT[:, mb, nsl],
                            start=(mb == 0), stop=(mb == NB - 1),
                        )
                return dps

            def emit_av(V, expT, recips, scrv):
                """OT = (V.T @ expT) * recip, streamed to DRAM scratch
                cb-major so stage_e's first rows unblock early."""
                for cb in range(CB):
                    csl = slice(cb * 128, (cb + 1) * 128)
                    for nh in range(N // NH):
                        nsl = slice(nh * NH, (nh + 1) * NH)
                        ps = psmm.tile([128, NH], f32, tag="mm")
                        if fp8:
                            for p in range(NB // 2):
                                nc.tensor.matmul(
                                    ps,
                                    V[:, 2 * p : 2 * p + 2, csl],
                                    expT[:, 2 * p : 2 * p + 2, nsl],
                                    start=(p == 0),
                                    stop=(p == NB // 2 - 1),
                                    perf_mode=DR,
                                )
                        else:
                            for mb in range(NB):
                                nc.tensor.matmul(
                                    ps, V[:, mb, csl], expT[:, mb, nsl],
                                    start=(mb == 0), stop=(mb == NB - 1),
                                )
                        ot = pool_row.tile([128, NH], MM, tag="row")
                        nc.vector.tensor_tensor(
                            ot, ps, recips[nh], op=mybir.AluOpType.mult
                        )
                        nc.sync.dma_start(scrv[csl, nsl], ot)

            def emit_prow(scr, ib):
                """P-row load, issued ~2 row-iterations ahead of its use so
                the DMA (and its queue wait) hides behind PE work."""
                pview = scr.rearrange("(i j) -> i j", j=C)
                prow = pool_row.tile([128, C], MM, tag="row")
                nc.sync.dma_start(prow, pview[ib * 128 : (ib + 1) * 128, :])
                return prow

            def emit_e_row(prow, b, ib):
                """One row-block of y = P @ proj_w + proj_b."""
                pt4a = pool_pt.tile([128, NH], MM, tag="pt4")
                pt4b = pool_pt.tile([128, NH], MM, tag="pt4")
                transpose_block(
                    prow,
                    [
                        pt4a.rearrange("p (c k) -> p c k", k=128),
                        pt4b[:, 0:256].rearrange("p (c k) -> p c k", k=128),
                    ],
                )
                ps1 = psmm.tile([128, NH], f32, tag="mm")
                ps2 = psmm.tile([128, NH], f32, tag="mm")
                for jb in range(CB):
                    pt = (pt4a if jb < 4 else pt4b)[
                        :, (jb % 4) * 128 : (jb % 4 + 1) * 128
                    ]
                    nc.tensor.matmul(
                        ps1, pt, PW[:, jb, 0:NH],
                        start=(jb == 0), stop=(jb == CB - 1),
                    )
                    nc.tensor.matmul(
                        ps2[:, : C - NH], pt, PW[:, jb, NH:C],
                        start=(jb == 0), stop=(jb == CB - 1),
                    )
                yrow = pool_row.tile([128, C], f32, tag="row")
                nc.vector.tensor_tensor(
                    yrow[:, 0:NH], ps1, pb[:, 0:NH], op=mybir.AluOpType.add
                )
                nc.vector.tensor_tensor(
                    yrow[:, NH:C], ps2[:, : C - NH], pb[:, NH:C],
                    op=mybir.AluOpType.add,
                )
                nc.sync.dma_start(y.ap()[b, ib * 128 : (ib + 1) * 128, :], yrow)

            # ---------------- emission schedule ----------------
            import contextlib
            _loop_n = int(os.environ.get("BLIP_LOOP", "0"))
            _loop_ctx = tc.For_i(0, _loop_n, 1) if _loop_n else contextlib.nullcontext()
            _loop_ctx.__enter__()

            # prologue: batch-0 x rows load before the big weight DMAs so the
            # PE starts transposing immediately; qkv_w streams in thirds
            # (q cols, k cols, v cols) so the first QK matmuls start after
            # 1/3 of the weight bytes.
            XT_cur = stage_a(0)
            w_view = qkv_w.rearrange("(cb p) o -> p cb o", p=128)
            pw_view = proj_w.rearrange("(cb p) o -> p cb o", p=128)
            for t in range(3):
                osl = slice(t * C, (t + 1) * C)
                for cb in range(CB):
                    nc.sync.dma_start(W[:, cb, osl], w_view[:, cb, osl])
            for cb in range(CB):
                nc.sync.dma_start(PW[:, cb], pw_view[:, cb])

            def make_qkv(XT):
                QT = pool_qt.tile([128, CB, N], AT, tag="QT")
                KT = pool_kt.tile([128, CB, N], AT, tag="KT")
                V = pool_v.tile([128, NB, C], AT, tag="V")
                return QT, KT, V

            qkv_cur = make_qkv(XT_cur)
            for ob in range(2 * CB):
                for nh in range(N // NH):
                    emit_qk_tile(XT_cur, qkv_cur[0], qkv_cur[1], ob, nh)
            for mb in range(NB):
                emit_v_chunk(XT_cur, qkv_cur[2], mb)

            for b in range(BPC):
                last = b + 1 >= BPC
                QT, KT, V = qkv_cur
                if not last:
                    XT_next = pool_xt.tile([128, CB, N], MM, tag="XT")
                    qkv_next = make_qkv(XT_next)

                # scores (both n-halves) with the next batch's x-row loads
                # and transposes woven in: each row DMA gets a scores tile
                # (~1.3us of PE) of cover before its transposes need it
                expT = pool_expt.tile([128, NB, N], AT, tag="expT")
                rows = [None] * NB
                for i, (nh, mb) in enumerate(
                    [(h, m) for h in range(N // NH) for m in range(NB)]
                ):
                    emit_scores_tile(QT, KT, expT, nh, mb)
                    if not last:
                        if i < NB:
                            rows[i] = emit_a_row_dma(b + 1, i)
                        if 1 <= i <= NB:
                            emit_a_row_transpose(XT_next, rows[i - 1], i - 1)

                # reciprocals right after the denominators, before the qk
                # tiles recycle their PSUM bufs
                recips = []
                for nh in range(N // NH):
                    dps = emit_denom(expT, nh)
                    rb = pool_rb.tile([128, NH], f32, tag="recipB")
                    nc.vector.reciprocal(rb, dps)
                    recips.append(rb)

                if not last:
                    for ob in range(2 * CB):
                        for nh in range(N // NH):
                            emit_qk_tile(XT_next, qkv_next[0], qkv_next[1], ob, nh)

                scr = pool_scr.tile([C * N], MM, tag="scr")
                scrv = scr.rearrange("(c n) -> c n", n=N)
                emit_av(V, expT, recips, scrv)

                prows = [None] * NB
                prows[0] = emit_prow(scr, 0)
                prows[1] = emit_prow(scr, 1)
                for ib in range(NB):
                    if not last:
                        emit_v_chunk(XT_next, qkv_next[2], ib)
                    emit_e_row(prows[ib], b, ib)
                    if ib + 2 < NB:
                        prows[ib + 2] = emit_prow(scr, ib + 2)

                if not last:
                    XT_cur, qkv_cur = XT_next, qkv_next

            _loop_ctx.__exit__(None, None, None)

    nc.compile()
    return nc


def _get_nc():
    mm_r = os.environ.get("BLIP_MM_DTYPE", "float32r") != "float32"
    # fp8 DoubleRow attention measures rel_err ~2.6e-2 on these inputs —
    # over the 2e-2 gate — so the bf16 core (rel_err 1.6e-3) is the default.
    fp8 = os.environ.get("BLIP_FP8", "0") == "1"
    key = ("nc", mm_r, fp8)
    if key not in _CACHE:
        _CACHE[key] = _build(mm_r, fp8)
    return _CACHE[key]


def kernel(x, qkv_w, qkv_b, proj_w, proj_b, _trace=False, _tmpdir=None):
    x = np.ascontiguousarray(np.asarray(x, dtype=np.float32))
    shared = {
        "qkv_w": np.ascontiguousarray(np.asarray(qkv_w, dtype=np.float32)),
        "qkv_b": np.ascontiguousarray(np.asarray(qkv_b, dtype=np.float32)),
        "proj_w": np.ascontiguousarray(np.asarray(proj_w, dtype=np.float32)),
        "proj_b": np.ascontiguousarray(np.asarray(proj_b, dtype=np.float32)),
    }
    nc = _get_nc()
    in_maps = [
        {"xs": x[c * BPC : (c + 1) * BPC], **shared} for c in range(NCORES)
    ]
    res = run_bass_kernel_spmd(
        nc, in_maps, core_ids=list(range(NCORES)),
        trace=_trace, tmpdir=_tmpdir,
        **({"trace_cores": [0]} if _trace else {}),
    )
    out = np.concatenate([res.results[c]["y"] for c in range(NCORES)], axis=0)
    if _trace:
        return out, res
    return out



# revision 4
# speedup vs baseline: 1.4301x; 1.4301x over previous
"""Trainium2 Bass kernel for nn_BlipAttention_75007308857568.

Single-head BLIP attention: B=32, N=1024, C=768, fp32.
  qkv = x @ qkv_w + qkv_b ; q,k,v split
  scores = q @ k.T / sqrt(C) ; attn = softmax(scores)
  out = attn @ v
  y = (out.swapaxes(1,2).reshape(B,N,C)) @ proj_w + proj_b

Sharding: data-parallel over batch B across 8 NeuronCores (4 batches/core).

Math restructuring (exact up to dropped softmax-invariant terms):
  q_n.k_m = x_n (Wq Wk^T) x_m^T + x_n.(Wq bk) + x_m.(Wk bq) + bq.bk
  The x_n.(Wq bk) and bq.bk terms are constant along the softmax axis (m)
  and drop out exactly. So with M = Wq @ Wk^T and w = Wk @ bq:
    scoresT[m,n] = (A_n . x_m)/sqrt(C) + (x_m . w)/sqrt(C),  A = x @ M
  K is never computed. The x.w term is applied as the per-partition bias of
  the exp activation (partition = m).

fp8 DoubleRow everywhere the error budget allows (e4m3, DR = 0.5 cyc/row,
256-deep contraction = 4x bf16 PE throughput), with residual compensation:
every operand X is carried as X8 = fp8(X), Xr = fp8(X - X8), and products
use 2-3 passes (X8*Y8 + Xr*Y8 + X8*Yr), dropping the tiny Xr*Yr term:
  A  = x @ (16 M)     3-pass fp8-DR    (M8/Mr precomputed on host)
  V  = x @ (16 Wv)    3-pass fp8-DR    (Wv8/Wvr on host; x16 keeps the
                                        small weights out of e4m3's
                                        subnormal range; the 16 cancels
                                        against a 16.0-constant in the
                                        softmax-denominator matmul)
  scoresT = X.A^T     3-pass fp8-DR    (per-batch A8/Ar, X8/Xr quantized
                                        on ACT/DVE from transpose PSUM)
  expT fp8 via ACT    exp(ps*S/16 + bias_m), bias_m = S/16*(x.16w) + EXPB
  OT = V^T @ expT     2-pass fp8-DR, * recip(16*sum e8) on DVE, + bv on ACT
                      (bv folded past the softmax: sum of weights == 1)
  proj in bf16        P round-trips DRAM scratch as bf16; c-major flat
                      scratch == the swapaxes+reshape permutation for free
Numpy-emulated end-to-end rel_err for this exact pipeline: 1.07e-2
(gate 2e-2; bf16 baseline 1.6e-3; plain fp8 without residuals 2.6e-2).

Engine use: PE does matmuls + transposes; ACT takes the fp8 quantize
copies (zero-add), exp, and per-partition bias adds; DVE takes the
residual subtracts, recip, normalize-multiply and proj bias adds. The
next batch's x-load/transpose/quantize and A/V matmuls are woven into the
current batch's attention to keep PE dense; the last batch interleaves
the projection into the AV stream.
"""

import math
import os

import numpy as np
import ml_dtypes

import concourse.bacc as bacc
import concourse.bass as bass
import concourse.mybir as mybir
import concourse.tile as tile

from concourse.bass_utils import run_bass_kernel_spmd
from concourse.masks import make_identity

B, N, C = 32, 1024, 768
NCORES = 8
BPC = B // NCORES  # batches per core
CB = C // 128      # 6 channel blocks
NB = N // 128      # 8 sequence blocks
NH = 512           # n-half width (PSUM bank limit for f32)
SCALE = 1.0 / math.sqrt(C)
S16 = SCALE / 16.0
EXPB = -2.0        # exp shift (max logit ~6.73 -> exp(4.73)=113 < 240)

F8NP = ml_dtypes.float8_e4m3   # trn2 e4m3: max 240, matches dt.float8e4
BFNP = ml_dtypes.bfloat16

_CACHE = {}


def _build():
    dt = mybir.dt
    MM = dt.float32r
    f32 = dt.float32
    F8 = dt.float8e4
    BF = dt.bfloat16
    DR = mybir.MatmulPerfMode.DoubleRow
    SUB = mybir.AluOpType.subtract
    ADD = mybir.AluOpType.add
    MUL = mybir.AluOpType.mult

    nc = bacc.Bacc("TRN2", target_bir_lowering=False, debug=False)

    # x in f32r (f32 bits) so the PE transposes run at 1.5 cyc/row
    xs = nc.dram_tensor("xs", [BPC, N, C], MM, kind="ExternalInput")
    # host-precomputed weight tensors (one-time transforms of qkv_w/proj_w)
    m8_d = nc.dram_tensor("m8", [C, C], F8, kind="ExternalInput")
    mr_d = nc.dram_tensor("mr", [C, C], F8, kind="ExternalInput")
    wv8_d = nc.dram_tensor("wv8", [C, C], F8, kind="ExternalInput")
    wvr_d = nc.dram_tensor("wvr", [C, C], F8, kind="ExternalInput")
    w8_d = nc.dram_tensor("w8", [C, 128], F8, kind="ExternalInput")
    pw_d = nc.dram_tensor("pw", [C, C], BF, kind="ExternalInput")
    qkv_b = nc.dram_tensor("qkv_b", [3 * C], f32, kind="ExternalInput")
    proj_b = nc.dram_tensor("proj_b", [C], f32, kind="ExternalInput")
    y = nc.dram_tensor("y", [BPC, N, C], f32, kind="ExternalOutput")

    with tile.TileContext(nc) as tc:
        with (
            tc.tile_pool(name="consts", bufs=1) as consts,
            tc.tile_pool(name="wts", bufs=1) as wts,
            tc.tile_pool(name="x8p", bufs=2) as x8p,
            tc.tile_pool(name="ap", bufs=1) as apool,
            tc.tile_pool(name="vp", bufs=1) as vpool,
            tc.tile_pool(name="ep", bufs=1) as epool,
            tc.tile_pool(name="bwp", bufs=2) as bwp,
            tc.tile_pool(name="rowp", bufs=4) as rowp,
            tc.tile_pool(name="otp", bufs=4) as otp,
            tc.tile_pool(name="rbp", bufs=2) as rbp,
            tc.tile_pool(name="ptp", bufs=2) as ptp,
            tc.tile_pool(name="scrp", bufs=2, space="DRAM") as scrp,
            tc.tile_pool(name="psmm", bufs=6, space="PSUM") as psmm,
            tc.tile_pool(name="pst", bufs=2, space="PSUM") as pst,
        ):
            # ---- constants ----
            ident_f = consts.tile([128, 128], f32, tag="ident_f")
            make_identity(nc, ident_f)
            ident = consts.tile([128, 128], MM, tag="ident")
            nc.vector.tensor_copy(ident, ident_f)
            ident_bf = consts.tile([128, 128], BF, tag="ident_bf")
            nc.vector.tensor_copy(ident_bf, ident_f)

            zero = consts.tile([128, 1], f32, tag="zero")
            nc.gpsimd.memset(zero, 0.0)

            ones11_f = consts.tile([1, 1], f32, tag="o11f")
            nc.gpsimd.memset(ones11_f, 1.0)
            ones11 = consts.tile([1, 1], BF, tag="o11")
            nc.vector.tensor_copy(ones11, ones11_f)

            # 16.0 constant cancels the x16 scaling of Wv in the softmax
            # denominator: recip(16*sum e8) * (16 V @ e8) == (V@e8)/sum e8
            ones16_f = consts.tile([128, 256], f32, tag="o16f")
            nc.gpsimd.memset(ones16_f, 16.0)
            ones16 = consts.tile([128, 256], F8, tag="o16")
            nc.gpsimd.tensor_copy(ones16, ones16_f)
            ones16_v = ones16.rearrange("p (k f) -> p k f", k=2)

            vbp = consts.tile([128, CB], f32, tag="vbp")
            nc.sync.dma_start(
                vbp, qkv_b.ap()[2 * C : 3 * C].rearrange("(cb p) -> p cb", p=128)
            )
            pb = consts.tile([128, C], f32, tag="pb")
            nc.sync.dma_start(pb, proj_b.ap()[None, :].to_broadcast([128, C]))

            # ---- weights (DMA only; all transforms were done on host) ----
            def ld3(name, dram, dtype):
                t = wts.tile([128, CB, C], dtype, tag=name)
                nc.sync.dma_start(t, dram.ap().rearrange("(cb p) o -> p cb o", p=128))
                return t

            w8 = wts.tile([128, CB, 128], F8, tag="w8")
            nc.sync.dma_start(w8, w8_d.ap().rearrange("(cb p) f -> p cb f", p=128))

            def emit_x_row_dma(b, nb):
                xrow = rowp.tile([128, C], MM, tag="xrow")
                nc.sync.dma_start(xrow, xs.ap()[b, nb * 128 : (nb + 1) * 128, :])
                return xrow

            def emit_x_row_quant(xrow, nb, X8, Xr):
                """PE-transpose one x row-block; quantize to fp8 + residual.
                X8 write on ACT (zero-add), residual subtract on DVE."""
                nsl = slice(nb * 128, (nb + 1) * 128)
                psA = pst.tile([128, NH], MM, tag="tp")
                for k in range(4):
                    nc.tensor.transpose(
                        psA[:, k * 128 : (k + 1) * 128],
                        xrow[:, k * 128 : (k + 1) * 128],
                        ident,
                    )
                srcA = psA.rearrange("p (c k) -> p c k", k=128)
                nc.scalar.add(X8[:, 0:4, nsl], srcA, zero[:, 0:1])
                nc.vector.tensor_tensor(
                    Xr[:, 0:4, nsl], srcA, X8[:, 0:4, nsl], op=SUB
                )
                psB = pst.tile([128, NH], MM, tag="tp")
                for k in range(2):
                    nc.tensor.transpose(
                        psB[:, k * 128 : (k + 1) * 128],
                        xrow[:, (4 + k) * 128 : (5 + k) * 128],
                        ident,
                    )
                srcB = psB[:, 0:256].rearrange("p (c k) -> p c k", k=128)
                nc.scalar.add(X8[:, 4:6, nsl], srcB, zero[:, 0:1])
                nc.vector.tensor_tensor(
                    Xr[:, 4:6, nsl], srcB, X8[:, 4:6, nsl], op=SUB
                )

            def emit_bw(X8, M8p_unused=None):
                """bias row: bw16[m] = sum_c w16[c] X8[c,m] (fp8-DR, padded
                stationary -> result on psum partition 0), bf16 [1,N] ->
                8 tiny matmul-transposes -> bwb[m, mb] = S16*bw16 + EXPB."""
                bw_sb = bwp.tile([1, N], BF, tag="bw_sb")
                for nh in range(2):
                    nsl = slice(nh * NH, (nh + 1) * NH)
                    ps = pst.tile([128, NH], f32, tag="tp")
                    for p in range(CB // 2):
                        nc.tensor.matmul(
                            ps, w8[:, 2 * p : 2 * p + 2, :],
                            X8[:, 2 * p : 2 * p + 2, nsl],
                            start=(p == 0), stop=(p == CB // 2 - 1),
                            perf_mode=DR,
                        )
                    nc.vector.tensor_copy(bw_sb[0:1, nsl], ps[0:1, :])
                psT = pst.tile([128, NH], f32, tag="tp")
                for mb in range(NB):
                    nc.tensor.matmul(
                        psT[:, mb : mb + 1],
                        bw_sb[0:1, mb * 128 : (mb + 1) * 128],
                        ones11, start=True, stop=True,
                    )
                bwb = bwp.tile([128, NB], f32, tag="bwb")
                nc.vector.tensor_scalar(
                    bwb, psT[:, 0:NB], S16, EXPB, op0=MUL, op1=ADD
                )
                return bwb

            def emit_a(X8, Xr, A8, Ar):
                """A = x @ 16M, 3-pass fp8-DR; A8/Ar quantize on ACT/DVE."""
                for ob in range(CB):
                    obsl = slice(ob * 128, (ob + 1) * 128)
                    ps0 = psmm.tile([128, NH], f32, tag="mm")
                    ps1 = psmm.tile([128, NH], f32, tag="mm")
                    for p in range(CB // 2):
                        ksl = slice(2 * p, 2 * p + 2)
                        st = M8[:, ksl, obsl]
                        nc.tensor.matmul(ps0, st, X8[:, ksl, 0:NH],
                                         start=(p == 0), stop=False, perf_mode=DR)
                        nc.tensor.matmul(ps1, st, X8[:, ksl, NH:N],
                                         start=(p == 0), stop=False, perf_mode=DR)
                        nc.tensor.matmul(ps0, st, Xr[:, ksl, 0:NH],
                                         start=False, stop=False, perf_mode=DR)
                        nc.tensor.matmul(ps1, st, Xr[:, ksl, NH:N],
                                         start=False, stop=False, perf_mode=DR)
                    for p in range(CB // 2):
                        ksl = slice(2 * p, 2 * p + 2)
                        st = Mr[:, ksl, obsl]
                        last = p == CB // 2 - 1
                        nc.tensor.matmul(ps0, st, X8[:, ksl, 0:NH],
                                         start=False, stop=last, perf_mode=DR)
                        nc.tensor.matmul(ps1, st, X8[:, ksl, NH:N],
                                         start=False, stop=last, perf_mode=DR)
                    nc.scalar.add(A8[:, ob, 0:NH], ps0, zero[:, 0:1])
                    nc.vector.tensor_tensor(Ar[:, ob, 0:NH], ps0,
                                            A8[:, ob, 0:NH], op=SUB)
                    nc.scalar.add(A8[:, ob, NH:N], ps1, zero[:, 0:1])
                    nc.vector.tensor_tensor(Ar[:, ob, NH:N], ps1,
                                            A8[:, ob, NH:N], op=SUB)

            def emit_v_mb(X8, Xr, V8, Vr, mb):
                """V[mb] = x[mb-block] @ 16Wv, 3-pass fp8-DR."""
                msl = slice(mb * 128, (mb + 1) * 128)
                psA = psmm.tile([128, NH], f32, tag="mm")
                psB = psmm.tile([128, NH], f32, tag="mm")
                for p in range(CB // 2):
                    ksl = slice(2 * p, 2 * p + 2)
                    st = X8[:, ksl, msl]
                    nc.tensor.matmul(psA, st, Wv8[:, ksl, 0:NH],
                                     start=(p == 0), stop=False, perf_mode=DR)
                    nc.tensor.matmul(psB[:, 0:256], st, Wv8[:, ksl, NH:C],
                                     start=(p == 0), stop=False, perf_mode=DR)
                    nc.tensor.matmul(psA, st, Wvr[:, ksl, 0:NH],
                                     start=False, stop=False, perf_mode=DR)
                    nc.tensor.matmul(psB[:, 0:256], st, Wvr[:, ksl, NH:C],
                                     start=False, stop=False, perf_mode=DR)
                for p in range(CB // 2):
                    ksl = slice(2 * p, 2 * p + 2)
                    st = Xr[:, ksl, msl]
                    last = p == CB // 2 - 1
                    nc.tensor.matmul(psA, st, Wv8[:, ksl, 0:NH],
                                     start=False, stop=last, perf_mode=DR)
                    nc.tensor.matmul(psB[:, 0:256], st, Wv8[:, ksl, NH:C],
                                     start=False, stop=last, perf_mode=DR)
                nc.scalar.add(V8[:, mb, 0:NH], psA, zero[:, 0:1])
                nc.vector.tensor_tensor(Vr[:, mb, 0:NH], psA,
                                        V8[:, mb, 0:NH], op=SUB)
                nc.scalar.add(V8[:, mb, NH:C], psB[:, 0:256], zero[:, 0:1])
                nc.vector.tensor_tensor(Vr[:, mb, NH:C], psB[:, 0:256],
                                        V8[:, mb, NH:C], op=SUB)

            def emit_scores_mb(X8, Xr, A8, Ar, e8, bwb, mb):
                """scoresT [mb, both n-halves], 3-pass fp8-DR + exp on ACT."""
                msl = slice(mb * 128, (mb + 1) * 128)
                ps0 = psmm.tile([128, NH], f32, tag="mm")
                ps1 = psmm.tile([128, NH], f32, tag="mm")
                for p in range(CB // 2):
                    ksl = slice(2 * p, 2 * p + 2)
                    st = X8[:, ksl, msl]
                    nc.tensor.matmul(ps0, st, A8[:, ksl, 0:NH],
                                     start=(p == 0), stop=False, perf_mode=DR)
                    nc.tensor.matmul(ps1, st, A8[:, ksl, NH:N],
                                     start=(p == 0), stop=False, perf_mode=DR)
                    nc.tensor.matmul(ps0, st, Ar[:, ksl, 0:NH],
                                     start=False, stop=False, perf_mode=DR)
                    nc.tensor.matmul(ps1, st, Ar[:, ksl, NH:N],
                                     start=False, stop=False, perf_mode=DR)
                for p in range(CB // 2):
                    ksl = slice(2 * p, 2 * p + 2)
                    st = Xr[:, ksl, msl]
                    last = p == CB // 2 - 1
                    nc.tensor.matmul(ps0, st, A8[:, ksl, 0:NH],
                                     start=False, stop=last, perf_mode=DR)
                    nc.tensor.matmul(ps1, st, A8[:, ksl, NH:N],
                                     start=False, stop=last, perf_mode=DR)
                nc.scalar.activation(
                    e8[:, mb, 0:NH], ps0, mybir.ActivationFunctionType.Exp,
                    scale=S16, bias=bwb[:, mb : mb + 1],
                )
                nc.scalar.activation(
                    e8[:, mb, NH:N], ps1, mybir.ActivationFunctionType.Exp,
                    scale=S16, bias=bwb[:, mb : mb + 1],
                )

            def emit_denom(e8, nh):
                nsl = slice(nh * NH, (nh + 1) * NH)
                dps = psmm.tile([128, NH], f32, tag="mm")
                for p in range(NB // 2):
                    nc.tensor.matmul(
                        dps, ones16_v, e8[:, 2 * p : 2 * p + 2, nsl],
                        start=(p == 0), stop=(p == NB // 2 - 1), perf_mode=DR,
                    )
                rb = rbp.tile([128, NH], f32, tag="rb")
                nc.vector.reciprocal(rb, dps)
                return rb

            def emit_av_cb(V8, Vr, e8, recips, scrv, cb):
                """OT[cb] both n-halves: 2-pass fp8-DR, DVE normalize,
                ACT +bv (exact: softmax weights sum to 1), bf16 scratch."""
                csl = slice(cb * 128, (cb + 1) * 128)
                ps0 = psmm.tile([128, NH], f32, tag="mm")
                ps1 = psmm.tile([128, NH], f32, tag="mm")
                for p in range(NB // 2):
                    ksl = slice(2 * p, 2 * p + 2)
                    st = V8[:, ksl, csl]
                    nc.tensor.matmul(ps0, st, e8[:, ksl, 0:NH],
                                     start=(p == 0), stop=False, perf_mode=DR)
                    nc.tensor.matmul(ps1, st, e8[:, ksl, NH:N],
                                     start=(p == 0), stop=False, perf_mode=DR)
                    st = Vr[:, ksl, csl]
                    last = p == NB // 2 - 1
                    nc.tensor.matmul(ps0, st, e8[:, ksl, 0:NH],
                                     start=False, stop=last, perf_mode=DR)
                    nc.tensor.matmul(ps1, st, e8[:, ksl, NH:N],
                                     start=False, stop=last, perf_mode=DR)
                for nh, ps in ((0, ps0), (1, ps1)):
                    nsl = slice(nh * NH, (nh + 1) * NH)
                    otm = otp.tile([128, NH], BF, tag="ot")
                    nc.vector.tensor_tensor(otm, ps, recips[nh], op=MUL)
                    ot = otp.tile([128, NH], BF, tag="ot")
                    nc.scalar.add(ot, otm, vbp[:, cb : cb + 1])
                    nc.sync.dma_start(scrv[csl, nsl], ot)

            def emit_prow(scr, ib):
                pview = scr.rearrange("(i j) -> i j", j=C)
                prow = rowp.tile([128, C], BF, tag="prow")
                nc.sync.dma_start(prow, pview[ib * 128 : (ib + 1) * 128, :])
                return prow

            def emit_pj_row(prow, b, ib):
                """One row-block of y = P @ proj_w + proj_b (bf16 core)."""
                pt4a = ptp.tile([128, NH], BF, tag="pt")
                pt4b = ptp.tile([128, NH], BF, tag="pt")
                psA = pst.tile([128, NH], BF, tag="tp")
                for k in range(4):
                    nc.tensor.transpose(
                        psA[:, k * 128 : (k + 1) * 128],
                        prow[:, k * 128 : (k + 1) * 128],
                        ident_bf,
                    )
                nc.vector.tensor_copy(pt4a, psA)
                psB = pst.tile([128, NH], BF, tag="tp")
                for k in range(2):
                    nc.tensor.transpose(
                        psB[:, k * 128 : (k + 1) * 128],
                        prow[:, (4 + k) * 128 : (5 + k) * 128],
                        ident_bf,
                    )
                nc.vector.tensor_copy(pt4b[:, 0:256], psB[:, 0:256])
                ps1 = psmm.tile([128, NH], f32, tag="mm")
                ps2 = psmm.tile([128, NH], f32, tag="mm")
                for jb in range(CB):
                    pt = (pt4a if jb < 4 else pt4b)[
                        :, (jb % 4) * 128 : (jb % 4 + 1) * 128
                    ]
                    nc.tensor.matmul(ps1, pt, PW[:, jb, 0:NH],
                                     start=(jb == 0), stop=(jb == CB - 1))
                    nc.tensor.matmul(ps2[:, 0:256], pt, PW[:, jb, NH:C],
                                     start=(jb == 0), stop=(jb == CB - 1))
                yrow = rowp.tile([128, C], f32, tag="yrow")
                nc.vector.tensor_tensor(yrow[:, 0:NH], ps1, pb[:, 0:NH], op=ADD)
                nc.vector.tensor_tensor(yrow[:, NH:C], ps2[:, 0:256],
                                        pb[:, NH:C], op=ADD)
                nc.sync.dma_start(y.ap()[b, ib * 128 : (ib + 1) * 128, :], yrow)

            # ---------------- emission schedule ----------------
            import contextlib
            _loop_n = int(os.environ.get("BLIP_LOOP", "0"))
            _loop_ctx = tc.For_i(0, _loop_n, 1) if _loop_n else contextlib.nullcontext()
            _loop_ctx.__enter__()

            def new_x8():
                X8t = x8p.tile([128, CB, N], F8, tag="X8")
                Xrt = x8p.tile([128, CB, N], F8, tag="Xr")
                return X8t, Xrt

            # prologue: batch-0 x rows first so PE starts transposing while
            # the (host-precomputed) weight tensors stream in
            X8c, Xrc = new_x8()
            rows0 = [emit_x_row_dma(0, nb) for nb in range(2)]
            M8 = ld3("M8", m8_d, F8)
            Mr = ld3("Mr", mr_d, F8)
            for nb in range(NB):
                if nb + 2 < NB:
                    rows0.append(emit_x_row_dma(0, nb + 2))
                emit_x_row_quant(rows0[nb], nb, X8c, Xrc)
            Wv8 = ld3("Wv8", wv8_d, F8)
            Wvr = ld3("Wvr", wvr_d, F8)
            PW = ld3("PW", pw_d, BF)

            A8 = apool.tile([128, CB, N], F8, tag="A8")
            Ar = apool.tile([128, CB, N], F8, tag="Ar")
            V8 = vpool.tile([128, NB, C], F8, tag="V8")
            Vr = vpool.tile([128, NB, C], F8, tag="Vr")

            bwb_c = emit_bw(X8c)
            emit_a(X8c, Xrc, A8, Ar)
            for mb in range(NB):
                emit_v_mb(X8c, Xrc, V8, Vr, mb)

            for b in range(BPC):
                last = b + 1 >= BPC
                if not last:
                    X8n, Xrn = new_x8()

                # scores with next batch's x load/transpose/quantize woven in
                e8 = epool.tile([128, NB, N], F8, tag="e8")
                rows = [None] * NB
                for mb in range(NB):
                    emit_scores_mb(X8c, Xrc, A8, Ar, e8, bwb_c, mb)
                    if not last:
                        rows[mb] = emit_x_row_dma(b + 1, mb)
                        if mb >= 1:
                            emit_x_row_quant(rows[mb - 1], mb - 1, X8n, Xrn)
                if not last:
                    emit_x_row_quant(rows[NB - 1], NB - 1, X8n, Xrn)

                recips = [emit_denom(e8, nh) for nh in range(2)]

                # next batch's bias row + A while this batch's softmax
                # normalizers settle on DVE
                if not last:
                    bwb_n = emit_bw(X8n)
                    emit_a(X8n, Xrn, A8, Ar)

                scr = scrp.tile([C * N], BF, tag="scr")
                scrv = scr.rearrange("(c n) -> c n", n=N)

                if not last:
                    for cb in range(CB):
                        emit_av_cb(V8, Vr, e8, recips, scrv, cb)
                    prows = [None] * NB
                    prows[0] = emit_prow(scr, 0)
                    prows[1] = emit_prow(scr, 1)
                    for ib in range(NB):
                        emit_v_mb(X8n, Xrn, V8, Vr, ib)
                        emit_pj_row(prows[ib], b, ib)
                        if ib + 2 < NB:
                            prows[ib + 2] = emit_prow(scr, ib + 2)
                    X8c, Xrc, bwb_c = X8n, Xrn, bwb_n
                else:
                    # epilogue: weave the projection into the AV stream.
                    # P row ib needs scratch channels < (ib+1)*96, i.e. AV
                    # blocks cb <= ceil((ib+1)*96/128)-1; lag 3 cbs for the
                    # DRAM round-trip.
                    ready = {0: [0], 1: [1], 2: [2, 3], 3: [4], 4: [5], 5: [6, 7]}
                    prows = {}
                    for cb in range(CB):
                        emit_av_cb(V8, Vr, e8, recips, scrv, cb)
                        for ib in ready[cb]:
                            prows[ib] = emit_prow(scr, ib)
                        if cb >= 3:
                            for ib in ready[cb - 3]:
                                emit_pj_row(prows[ib], b, ib)
                    for cb in range(CB - 3, CB):
                        for ib in ready[cb]:
                            emit_pj_row(prows[ib], b, ib)

            _loop_ctx.__exit__(None, None, None)

    nc.compile()
    return nc


def _get_nc():
    if "nc" not in _CACHE:
        _CACHE["nc"] = _build()
    return _CACHE["nc"]


def _prep_weights(qkv_w, qkv_b, proj_w):
    """Host-side one-time weight transforms (fp8+residual pairs)."""
    Wq, Wk, Wv = qkv_w[:, :C], qkv_w[:, C : 2 * C], qkv_w[:, 2 * C :]
    bq = qkv_b[:C]

    def split8(a):
        a8 = a.astype(F8NP)
        return a8, (a - a8.astype(np.float32)).astype(F8NP)

    M16 = 16.0 * (Wq @ Wk.T)          # [c1, c2]
    m8, mr = split8(M16)
    wv8, wvr = split8(16.0 * Wv)
    w16 = 16.0 * (Wk @ bq)            # [c]
    w8 = np.zeros((C, 128), dtype=F8NP)
    w8[:, 0] = w16.astype(F8NP)
    pw = proj_w.astype(BFNP)
    return {"m8": m8, "mr": mr, "wv8": wv8, "wvr": wvr, "w8": w8, "pw": pw}


def kernel(x, qkv_w, qkv_b, proj_w, proj_b, _trace=False, _tmpdir=None):
    x = np.ascontiguousarray(np.asarray(x, dtype=np.float32))
    qkv_w = np.ascontiguousarray(np.asarray(qkv_w, dtype=np.float32))
    qkv_b = np.ascontiguousarray(np.asarray(qkv_b, dtype=np.float32))
    proj_w = np.ascontiguousarray(np.asarray(proj_w, dtype=np.float32))
    proj_b = np.ascontiguousarray(np.asarray(proj_b, dtype=np.float32))

    shared = _prep_weights(qkv_w, qkv_b, proj_w)
    shared["qkv_b"] = qkv_b
    shared["proj_b"] = proj_b

    nc = _get_nc()
    in_maps = [
        {"xs": x[c * BPC : (c + 1) * BPC], **shared} for c in range(NCORES)
    ]
    res = run_bass_kernel_spmd(
        nc, in_maps, core_ids=list(range(NCORES)),
        trace=_trace, tmpdir=_tmpdir,
        **({"trace_cores": [0]} if _trace else {}),
    )
    out = np.concatenate([res.results[c]["y"] for c in range(NCORES)], axis=0)
    if _trace:
        return out, res
    return out
